# revision 1
# baseline (speedup 1.0000x reference)
"""Trainium2 Bass kernel for GCL contrastive-scoring GNN message passing.

Computation (see the reference):
  h   = x @ W + b                      [N, H]
  q   = sigmoid(h)                     [N, H]
  k_p = normalize(segsum(pw*h))        [Np, H]
  k_n = segsum(pw*q)                   [Np, H]
  att = exp(k_p @ k_p.T / T)           [Np, Np]
  pos = (att * A_P) @ k_n ; neg = att @ k_n
  loss = mean(-log(q.pos[par]) + log(q.neg[par]))

Sharding: nodes are assigned to the core owning their parent block
(parent p -> core p // 512).  Within a core, nodes are grouped into 4
"windows" of 128 parents each and padded to a uniform per-window quota so
the SPMD program is identical across all 8 cores.  Each core computes its
[512, H] k_p / k_n band, bands are AllGathered, each core computes its
512 rows of att / pos_msg / neg_msg (i = own band, j = all parents), and
the scores for its own nodes.  Segment-sum and the parent->node gather
are expressed as matmuls against host-built one-hot matrices (pw folded
into the segment one-hot).  Per-core partial loss sums are combined on
host.  Inputs are laid out host-side so each device load is one large
contiguous DMA.
"""

import numpy as np
import ml_dtypes

import concourse.bass as bass
import concourse.bacc as bacc
import concourse.mybir as mybir
import concourse.tile as tile
from concourse import bass_utils

F32 = mybir.dt.float32
F32R = mybir.dt.float32r
BF16 = mybir.dt.bfloat16
AF = mybir.ActivationFunctionType
ALU = mybir.AluOpType

NCORES = 8
NP = 4096          # parents
BAND = 512         # parents per core
NW = 4             # windows (128 parents) per core
D = 512
H = 512
KT = 4             # 128-tiles along D/H
NJ = NP // 128     # 32 j tiles
TEMP_SCALE = 2.0   # 1 / TEMPERATURE
EPS = 1e-12
NB16 = ml_dtypes.bfloat16
QCH = 8            # q-reload tiles per DMA in phase 3


# ----------------------------------------------------------------- host prep

def prep_inputs(x, node_to_par, p_weight, A_P, W, b):
    x = np.asarray(x, np.float32)
    par = np.asarray(node_to_par).astype(np.int64)
    pw = np.asarray(p_weight, np.float32)
    A_P = np.asarray(A_P, np.float32)
    W = np.asarray(W, np.float32)
    b = np.asarray(b, np.float32)
    N = x.shape[0]

    # group nodes by (core, window); window quota uniform over all groups
    grp = par // 128                       # [N] in [0, 32)
    order = np.argsort(grp, kind="stable")
    grp_sorted = grp[order]
    bounds = np.searchsorted(grp_sorted, np.arange(NCORES * NW + 1))
    counts = np.diff(bounds)
    Q0 = int(np.ceil(max(1, counts.max()) / 128) * 128)   # per-window quota
    QT = Q0 // 128
    T = NW * QT
    n_pad = NW * Q0

    bias_nonzero = bool(np.any(b != 0.0))

    per_core = []
    for c in range(NCORES):
        slots = np.full(n_pad, -1, np.int64)
        for w in range(NW):
            g = c * NW + w
            lo, hi = bounds[g], bounds[g + 1]
            slots[w * Q0: w * Q0 + (hi - lo)] = order[lo:hi]
        valid = slots >= 0
        sc = np.where(valid, slots, 0)

        # xs[t, p, 0:4, n] = x[node(t,n), k*128+p];  xs[t, p, 4, m] = seg 1-hot
        x_sel = np.where(valid[:, None], x[sc], 0.0).astype(np.float32)
        xs = np.empty((T, 128, KT + 1, 128), NB16)
        xs[:, :, :KT, :] = x_sel.reshape(T, 128, KT, 128).transpose(0, 3, 2, 1)

        wslot = np.arange(n_pad) // Q0
        cols = np.where(valid, par[sc] - c * BAND - wslot * 128, 0)
        rows = np.arange(n_pad)

        seg = np.zeros((n_pad, 128), np.float32)
        seg[rows[valid], cols[valid]] = pw[sc[valid]]
        xs[:, :, KT, :] = seg.reshape(T, 128, 128)

        # gather one-hot, [parent, node] per tile, laid out [p, t*128+n]
        gath = np.zeros((n_pad, 128), np.float32)
        gath[rows, cols] = 1.0
        gathA = np.ascontiguousarray(
            gath.reshape(T, 128, 128).transpose(2, 0, 1).reshape(128, T * 128)
        ).astype(NB16)

        maskT = np.ascontiguousarray(valid.reshape(T, 128).T).astype(np.float32)

        # A_P transposed band, laid out [p, jt*512+i]  (p = j within tile)
        A_PT = A_P[c * BAND:(c + 1) * BAND, :].T            # [4096 j, 512 i]
        aptA = np.ascontiguousarray(
            A_PT.reshape(NJ, 128, BAND).transpose(1, 0, 2).reshape(128, NJ * BAND)
        ).astype(NB16)

        per_core.append({
            "xs": np.ascontiguousarray(
                xs.reshape(T, 128, (KT + 1) * 128).transpose(1, 0, 2)
            ).reshape(128, T * (KT + 1) * 128),
            "gathA": gathA, "maskT": maskT, "aptA": aptA,
            "W4": W.reshape(KT, 128, H).astype(NB16),
            "ident": np.eye(128, dtype=np.float32),
            **({"bvec": b.reshape(1, H).astype(NB16)} if bias_nonzero else {}),
        })
    meta = {"N": N, "T": T, "QT": QT, "n_pad": n_pad,
            "bias_nonzero": bias_nonzero}
    return per_core, meta


# ------------------------------------------------------------ device program

def build_program(T, QT, bias_nonzero, stage=4, reps=1, no_coll=False):
    nc = bacc.Bacc("TRN2", target_bir_lowering=False, debug=False,
                   num_devices=NCORES)

    i_xs = nc.dram_tensor("xs", [128, T * (KT + 1) * 128], BF16,
                          kind="ExternalInput")
    i_gath = nc.dram_tensor("gathA", [128, T * 128], BF16, kind="ExternalInput")
    i_mask = nc.dram_tensor("maskT", [128, T], F32, kind="ExternalInput")
    i_apt = nc.dram_tensor("aptA", [128, NJ * BAND], BF16, kind="ExternalInput")
    i_w = nc.dram_tensor("W4", [KT, 128, H], BF16, kind="ExternalInput")
    i_id = nc.dram_tensor("ident", [128, 128], F32, kind="ExternalInput")
    if bias_nonzero:
        i_b = nc.dram_tensor("bvec", [1, H], BF16, kind="ExternalInput")
    o_loss = nc.dram_tensor("loss_part", [128, 1], F32, kind="ExternalOutput")

    with tile.TileContext(nc) as tc:
        with (
            tc.tile_pool(name="const", bufs=1) as constp,
            tc.tile_pool(name="bands", bufs=1) as bandp,
            tc.tile_pool(name="strm", bufs=3) as strm,
            tc.tile_pool(name="strm2", bufs=2) as strm2,
            tc.tile_pool(name="dram", bufs=1, space="DRAM") as dram,
        ):
            # constants / resident tensors
            w_sb = constp.tile([128, KT * H], BF16, tag="w")
            for k in range(KT):
                nc.sync.dma_start(w_sb[:, k * H:(k + 1) * H], i_w[k])
            ident = constp.tile([128, 128], F32, tag="ident")
            nc.sync.dma_start(ident[:], i_id[:])
            gath_sb = constp.tile([128, T * 128], BF16, tag="gath")
            nc.sync.dma_start(gath_sb[:], i_gath[:])
            apt_sb = constp.tile([128, NJ * BAND], BF16, tag="apt")
            nc.sync.dma_start(apt_sb[:], i_apt[:])
            if bias_nonzero:
                bias_sb = constp.tile([1, H], BF16, tag="bias")
                nc.sync.dma_start(bias_sb[:], i_b[:])
                ones1 = constp.tile([1, 128], BF16, tag="ones1")
                nc.vector.memset(ones1[:], 1.0)

            # persistent SBUF
            kpj_all = bandp.tile([128, NJ * KT * 128], BF16, tag="kpj_all")
            knj_all = bandp.tile([128, NJ * H], BF16, tag="knj_all")
            kp_band = bandp.tile([128, NW * H], F32, tag="kp_band")
            kn_band = bandp.tile([128, NW * H], BF16, tag="kn_band")
            kpT_band = bandp.tile([128, KT * BAND], BF16, tag="kpT_band")
            msgs = bandp.tile([128, NW * 1024], BF16, tag="msgs")
            sc_pos = bandp.tile([128, T], F32, tag="sc_pos")
            sc_neg = bandp.tile([128, T], F32, tag="sc_neg")

            qspill = dram.tile([128, T * H], mybir.dt.float8e4)
            agin_kp = dram.tile([NW, 128, KT * 128], BF16)
            agout_kp = dram.tile([NCORES, NW, 128, KT * 128], BF16)
            agin_kn = dram.tile([NW, 128, H], BF16)
            agout_kn = dram.tile([NCORES, NW, 128, H], BF16)

            for _rep in range(reps):
              # ---------------- phase 1: encoder + segment sums ---------------
              ps1 = tc.tile_pool(name=f"ps1_{_rep}", bufs=2, space="PSUM")
              psA = ps1.__enter__()
              XW = (KT + 1) * 128
              XCH = 4
              for w in range(NW):
                  ps_kp = psA.tile([128, H], F32, tag="kp")
                  ps_kn = psA.tile([128, H], F32, tag="kn")
                  for ti in range(QT):
                      t = w * QT + ti
                      if ti % XCH == 0:
                          nxc = min(XCH, QT - ti)
                          xch = strm.tile([128, XCH * XW], BF16, tag="xch",
                                          bufs=2)
                          nc.sync.dma_start(
                              xch[:, :nxc * XW],
                              i_xs[:, t * XW:(t + nxc) * XW])
                      if ti % XCH == 0 or True:
                          xt = xch[:, (ti % XCH) * XW:(ti % XCH + 1) * XW]

                      ps_h = psA.tile([128, H], F32, tag="h")
                      if bias_nonzero:
                          nc.tensor.matmul(ps_h[:], ones1[:], bias_sb[:],
                                           start=True, stop=False)
                      for k in range(KT):
                          nc.tensor.matmul(
                              ps_h[:], xt[:, k * 128:(k + 1) * 128],
                              w_sb[:, k * H:(k + 1) * H],
                              start=(k == 0 and not bias_nonzero),
                              stop=(k == KT - 1))

                      hq = strm.tile([128, 2 * H], BF16, tag="hq")
                      nc.vector.tensor_copy(hq[:, 0:H], ps_h[:])
                      nc.scalar.activation(hq[:, H:2 * H], ps_h[:], AF.Sigmoid)
                      if ti % XCH == 0:
                          q8 = strm.tile([128, XCH * H], mybir.dt.float8e4,
                                         tag="q8", bufs=2)
                      nc.vector.tensor_copy(
                          q8[:, (ti % XCH) * H:(ti % XCH + 1) * H],
                          hq[:, H:2 * H])
                      if ti % XCH == nxc - 1 or ti == QT - 1:
                          nb = (ti % XCH) + 1
                          t0q = t - nb + 1
                          nc.scalar.dma_start(
                              qspill[:, t0q * H:(t0q + nb) * H],
                              q8[:, :nb * H])

                      oh = xt[:, KT * 128:(KT + 1) * 128]
                      nc.tensor.matmul(ps_kp[:], oh, hq[:, 0:H],
                                       start=(ti == 0), stop=(ti == QT - 1))
                      nc.tensor.matmul(ps_kn[:], oh, hq[:, H:2 * H],
                                       start=(ti == 0), stop=(ti == QT - 1))

                  # normalize k_p rows; copy bands out of PSUM
                  kp_raw = strm2.tile([128, H], F32, tag="kp_raw")
                  nc.vector.tensor_copy(kp_raw[:], ps_kp[:])
                  ssq = strm2.tile([128, 1], F32, tag="ssq")
                  tmp = strm2.tile([128, H], F32, tag="nrm_tmp")
                  nc.scalar.activation(tmp[:], kp_raw[:], AF.Square,
                                       accum_out=ssq[:])
                  nrm = strm2.tile([128, 1], F32, tag="nrm")
                  nc.scalar.activation(nrm[:], ssq[:], AF.Sqrt)
                  nc.vector.tensor_scalar_max(nrm[:], nrm[:], EPS)
                  rinv = strm2.tile([128, 1], F32, tag="rinv")
                  nc.vector.reciprocal(rinv[:], nrm[:])
                  nc.scalar.mul(kp_band[:, w * H:(w + 1) * H], kp_raw[:], rinv[:])
                  nc.vector.tensor_copy(kn_band[:, w * H:(w + 1) * H], ps_kn[:])

              if stage >= 2:
                  # transpose normalized k_p band -> kpT_band [h, p]
                  for w in range(NW):
                      for s in range(KT):
                          ps_t = psA.tile([128, 128], F32, tag="tr", bufs=2)
                          nc.tensor.transpose(
                              ps_t[:],
                              kp_band[:, w * H + s * 128: w * H + (s + 1) * 128],
                              ident[:])
                          nc.vector.tensor_copy(
                              kpT_band[:, s * BAND + w * 128:
                                       s * BAND + (w + 1) * 128],
                              ps_t[:])
              ps1.__exit__(None, None, None)

              if stage >= 2:
                  # ---------------- all-gather bands ----------------
                  # agin_kp[w][p, s*128+q] = kpT_band[p, s*512 + w*128 + q]
                  kpT_r = kpT_band.rearrange("p (s q) -> p s q", s=KT)
                  for w in range(NW):
                      nc.sync.dma_start(
                          agin_kp[w],
                          kpT_r[:, :, w * 128:(w + 1) * 128])
                  for w in range(NW):
                      nc.sync.dma_start(agin_kn[w],
                                        kn_band[:, w * H:(w + 1) * H])
                  if no_coll:
                      for b in range(NCORES):
                          nc.sync.dma_start(agout_kp[b], agin_kp[:])
                          nc.sync.dma_start(agout_kn[b], agin_kn[:])
                  else:
                      nc.gpsimd.collective_compute(
                          "AllGather", ALU.bypass,
                          replica_groups=[list(range(NCORES))],
                          ins=[agin_kp.opt()], outs=[agout_kp.opt()])
                      nc.gpsimd.collective_compute(
                          "AllGather", ALU.bypass,
                          replica_groups=[list(range(NCORES))],
                          ins=[agin_kn.opt()], outs=[agout_kn.opt()])

              if stage >= 3:
                  # ---------------- phase 2: attention + messages --------------
                  # i = own band (512 parents), j = all parents; 2 passes over j
                  # (one per 256-wide half of i) so PSUM holds 4 accumulators.
                  ps2 = tc.tile_pool(name=f"ps2_{_rep}", bufs=2, space="PSUM")
                  psB = ps2.__enter__()
                  for ihalf in range(2):
                      acc = [psB.tile([128, H], F32, tag=f"acc{g}", bufs=1,
                                      name=f"acc{ihalf}{g}")
                             for g in range(4)]
                      for jg in range(NJ):
                          bsel, wsel = jg // NW, jg % NW
                          kptj = kpj_all[:, jg * KT * 128:(jg + 1) * KT * 128]
                          knj = knj_all[:, jg * H:(jg + 1) * H]
                          if ihalf == 0:
                              nc.sync.dma_start(kptj, agout_kp[bsel, wsel])
                              nc.sync.dma_start(knj, agout_kn[bsel, wsel])

                          ps_att = psB.tile([128, 256], F32, tag="att")
                          for s in range(KT):
                              nc.tensor.matmul(
                                  ps_att[:],
                                  kpj_all[:, jg * KT * 128 + s * 128:
                                          jg * KT * 128 + (s + 1) * 128],
                                  kpT_band[:, s * BAND + ihalf * 256:
                                           s * BAND + ihalf * 256 + 256],
                                  start=(s == 0), stop=(s == KT - 1))
                          att_bf = strm.tile([128, 256], BF16, tag="att_bf")
                          nc.scalar.activation(att_bf[:], ps_att[:], AF.Exp,
                                               scale=TEMP_SCALE)
                          wpos = strm.tile([128, 256], BF16, tag="wpos")
                          nc.vector.tensor_mul(
                              wpos[:], att_bf[:],
                              apt_sb[:, jg * BAND + ihalf * 256:
                                     jg * BAND + ihalf * 256 + 256])
                          for i2 in range(2):
                              nc.tensor.matmul(acc[i2][:],
                                               wpos[:, i2 * 128:(i2 + 1) * 128],
                                               knj,
                                               start=(jg == 0),
                                               stop=(jg == NJ - 1))
                              nc.tensor.matmul(acc[2 + i2][:],
                                               att_bf[:, i2 * 128:(i2 + 1) * 128],
                                               knj,
                                               start=(jg == 0),
                                               stop=(jg == NJ - 1))
                      for i2 in range(2):
                          g = ihalf * 2 + i2   # global i-sub == window index
                          nc.scalar.copy(msgs[:, g * 1024:g * 1024 + 512],
                                         acc[i2][:])
                          nc.scalar.copy(msgs[:, g * 1024 + 512:(g + 1) * 1024],
                                         acc[2 + i2][:])
                  ps2.__exit__(None, None, None)

              if stage >= 4:
                  # ---------------- phase 3: per-node scores ----------------
                  ps3 = tc.tile_pool(name=f"ps3_{_rep}", bufs=2, space="PSUM")
                  psC = ps3.__enter__()
                  for t0 in range(0, T, QCH):
                      nq = min(QCH, T - t0)
                      qch = strm.tile([128, QCH * H], mybir.dt.float8e4, tag="qch",
                                          bufs=2)
                      nc.sync.dma_start(qch[:, :nq * H],
                                        qspill[:, t0 * H:(t0 + nq) * H])
                      for ti in range(nq):
                          t = t0 + ti
                          w = t // QT
                          qt = qch[:, ti * H:(ti + 1) * H]
                          gt = gath_sb[:, t * 128:(t + 1) * 128]
                          ps_gp = psC.tile([128, H], F32, tag="gp")
                          nc.tensor.matmul(ps_gp[:], gt,
                                           msgs[:, w * 1024:w * 1024 + 512],
                                           start=True, stop=True)
                          ps_gn = psC.tile([128, H], F32, tag="gn")
                          nc.tensor.matmul(ps_gn[:], gt,
                                           msgs[:, w * 1024 + 512:(w + 1) * 1024],
                                           start=True, stop=True)
                          prod = strm.tile([128, H], BF16, tag="prod")
                          nc.vector.tensor_mul(prod[:], qt, ps_gp[:])
                          dump = strm.tile([128, H], BF16, tag="dump")
                          nc.scalar.activation(dump[:], prod[:], AF.Copy,
                                               accum_out=sc_pos[:, t:t + 1])
                          prod2 = strm.tile([128, H], BF16, tag="prod")
                          nc.vector.tensor_mul(prod2[:], qt, ps_gn[:])
                          dump2 = strm.tile([128, H], BF16, tag="dump")
                          nc.scalar.activation(dump2[:], prod2[:], AF.Copy,
                                               accum_out=sc_neg[:, t:t + 1])
                  ps3.__exit__(None, None, None)

                  # loss = sum(mask * (ln(neg) - ln(pos)))
                  lpos = bandp.tile([128, T], F32, tag="lpos")
                  nc.scalar.activation(lpos[:], sc_pos[:], AF.Ln)
                  lneg = bandp.tile([128, T], F32, tag="lneg")
                  nc.scalar.activation(lneg[:], sc_neg[:], AF.Ln)
                  dl = bandp.tile([128, T], F32, tag="dl")
                  nc.vector.tensor_sub(dl[:], lneg[:], lpos[:])
                  mk = bandp.tile([128, T], F32, tag="mk")
                  nc.sync.dma_start(mk[:], i_mask[:])
                  nc.vector.tensor_mul(dl[:], dl[:], mk[:])
                  lsum = bandp.tile([128, 1], F32, tag="lsum")
                  nc.vector.tensor_reduce(lsum[:], dl[:], mybir.AxisListType.X,
                                          ALU.add)
                  nc.sync.dma_start(o_loss[:], lsum[:])
              elif stage == 1:
                  dbg = strm2.tile([128, 1], F32, tag="dbg")
                  nc.vector.tensor_copy(dbg[:], kp_band[:, 0:1])
                  nc.sync.dma_start(o_loss[:], dbg[:])
              elif stage == 2:
                  tmpld = strm2.tile([128, 1], BF16, tag="tmpld")
                  nc.sync.dma_start(tmpld[:], agout_kn[7, 0][:, 0:1])
                  dbg = strm2.tile([128, 1], F32, tag="dbg")
                  nc.vector.tensor_copy(dbg[:], tmpld[:])
                  nc.sync.dma_start(o_loss[:], dbg[:])
              elif stage == 3:
                  dbg = strm2.tile([128, 1], F32, tag="dbg")
                  nc.vector.tensor_copy(dbg[:], msgs[:, 0:1])
                  nc.sync.dma_start(o_loss[:], dbg[:])

    nc.compile()
    return nc


_CACHE = {}


def get_compiled(T, QT, bias_nonzero, stage=4, reps=1):
    key = (T, QT, bias_nonzero, stage, reps)
    if key not in _CACHE:
        _CACHE[key] = build_program(T, QT, bias_nonzero, stage, reps)
    return _CACHE[key]


def make_in_maps(per_core):
    return [dict(d) for d in per_core]


def kernel(x, node_to_par, p_weight, A_P, W, b):
    per_core, meta = prep_inputs(x, node_to_par, p_weight, A_P, W, b)
    nc = get_compiled(meta["T"], meta["QT"], meta["bias_nonzero"])
    res = bass_utils.run_bass_kernel_spmd(
        nc, make_in_maps(per_core), core_ids=list(range(NCORES)))
    total = np.float64(0.0)
    for c in range(NCORES):
        total += np.asarray(res.results[c]["loss_part"], np.float64).sum()
    return np.float32(total / meta["N"])

